# revision 8
# baseline (speedup 1.0000x reference)
"""Corr1d-x-group cost-volume kernel for Trainium2 (8 NeuronCores, SPMD).

Data-parallel over batch N=8: core i processes batch i.

Per core (inputs [16, 256, 512], output [108, 256, 512]):
  out[g*27+ch, h, w] = 0.25 * sum_c f1[g*4+c, h, w] * f2[g*4+c, h, w+ch-23]
with zero padding outside w in [0, 512).

v3 design (bottleneck: the shift-product stream on VectorE, capped at DVE
2x_1P mode ~= (58 + FD/2)/0.96 ns per op):
  - Host pre-casts inputs to f16 and PRE-ARRANGES them into per-block SBUF
    images (zero pads + dual parity copies of f2 baked in), so every load
    is a contiguous DMA. Output stored f16; host un-permutes and upcasts.
  - SBUF layout per 32-row h-block: partitions = (c4, h32) = 128.
  - DVE work fused into 4 chunk ops per block: each computes 6-7 same-parity
    shifts x 4 groups in ONE tensor_tensor (overlapping strided view of the
    padded f2 tile; f1 broadcast along the shift dim with stride 0).
    Dual parity copies keep every row start 4B-aligned for 2x_1P mode.
  - Channel reduction via TensorE: 4 col-tiled matmuls per shift with a
    constant [128, 32] block-diagonal 0.25 weight -> fp32 PSUM; 4 shifts
    share one 4-bank PSUM tile.
  - ScalarE evacuates PSUM -> f16 SBUF staging (cast in ACTIVATE). Staging
    is in shift-production order (evens then odds); two strided stores per
    block de-interleave channels back to DRAM order.
"""

import os
import numpy as np

import concourse.bass as bass
import concourse.bacc as bacc
import concourse.mybir as mybir
import concourse.tile as tile
from concourse import bass_utils

N, C, H, W = 8, 16, 256, 512
G = 4
TOP_CH = 27
OUT_CH = G * TOP_CH  # 108
HB = 32  # h rows per block; 4 channels * 32 rows = 128 partitions
NBLK = H // HB
PADL = 24  # f2 even tile: column = w + PADL within each 544-col group chunk
F2W = PADL + W + 8  # 544 columns per group chunk
PSH = 4  # shifts per PSUM tile (4 banks); last tile holds 3

# Shift production order: even channels (from the odd-parity tile, aligned
# starts 0,2,..,26) then odd channels (from the even-parity tile, starts
# 2,4,..,26). Produced index k -> channel: k<14 -> 2k; k>=14 -> 2(k-14)+1.
# DVE chunks: (source, start_col, n_shifts); source 'o' = f2o, 'e' = f2e.
CHUNKS = [("o", 0, 7), ("o", 14, 7), ("e", 2, 7), ("e", 16, 2)]
# Shifts offloaded to GpSimd (Pool) as single-shift ops: f2e starts 20..26
# = channels 19,21,23,25 (produced k = 23..26).
POOL_STARTS = [20, 22, 24, 26]

_CACHED = {}


def _reduction_weights() -> np.ndarray:
    # lhsT [K=(c, h32)=128, M=h32=32]: sums the 4 channels of a group and
    # applies the 1/sumelems scale.
    w = np.zeros((128, 32), np.float16)
    for c in range(G):
        for hh in range(HB):
            w[c * HB + hh, hh] = 0.25
    return w


def _ap(base, offset_elems, dims):
    """Raw AP on base's tensor: dims = [[stride, count], ...] in elements."""
    return bass.AP(tensor=base.tensor, offset=base.offset + offset_elems, ap=dims)


def _build_program() -> bass.Bass:
    # Bacc (not raw Bass): its compile() splits multi-sem sync waits, which
    # TRN2 hardware limits to one per instruction.
    nc = bacc.Bacc(
        "TRN2",
        target_bir_lowering=False,
        debug=False,
        enable_asserts=False,
        num_devices=N,
    )
    f16 = mybir.dt.float16
    f32 = mybir.dt.float32

    l_blk = nc.dram_tensor("l_blk", [NBLK, 128, G * W], f16, kind="ExternalInput")
    r_e = nc.dram_tensor("r_e", [NBLK, 128, G * F2W], f16, kind="ExternalInput")
    r_o = nc.dram_tensor("r_o", [NBLK, 128, G * F2W], f16, kind="ExternalInput")
    w_red = nc.dram_tensor("w_red", [128, 32], f16, kind="ExternalInput")
    # out[ib][g*32+h][ch*512+w] = out[g*27+ch, ib*32+h, w] (channel order).
    out = nc.dram_tensor("out", [NBLK, 128, TOP_CH * W], f16, kind="ExternalOutput")
    OBW = TOP_CH * W  # 13824 staging columns per partition

    with tile.TileContext(nc) as tc:
        with (
            tc.tile_pool(name="wpool", bufs=1) as wpool,
            tc.tile_pool(name="inpool", bufs=2) as inpool,
            tc.tile_pool(name="prodpool", bufs=3) as prodpool,
            tc.tile_pool(name="obpool", bufs=2) as obpool,
            tc.tile_pool(name="psumpool", bufs=2, space="PSUM") as psumpool,
        ):
            wt = wpool.tile([128, 32], f16)
            nc.sync.dma_start(wt[:], w_red[:])

            for ib in range(NBLK):
                # f2e (only needed by the 3rd chunk onward) rides the idle
                # Activation HWDGE ring; the SP ring serves f1 + f2o so the
                # first chunk starts as early as possible.
                f1t = inpool.tile([128, G * W], f16, tag="f1t")
                nc.sync.dma_start(f1t[:], l_blk.ap()[ib : ib + 1])
                f2o = inpool.tile([128, G * F2W], f16, tag="f2o")
                nc.sync.dma_start(f2o[:], r_o.ap()[ib : ib + 1])
                f2e = inpool.tile([128, G * F2W], f16, tag="f2e")
                nc.scalar.dma_start(f2e[:], r_e.ap()[ib : ib + 1])

                ob = obpool.tile([128, OBW], f16, tag="ob")
                f1t3 = f1t[:].rearrange("p (g w) -> p g w", g=G)
                f2e3 = f2e[:].rearrange("p (g w) -> p g w", g=G)

                # Pool (GpSimd) single-shift products, issued first so they
                # overlap the whole block's DVE chunks.
                pool_prods = []
                for col in POOL_STARTS:
                    pprod = prodpool.tile([128, G * W], f16, tag="pprod", bufs=5)
                    pprod3 = pprod[:].rearrange("p (g w) -> p g w", g=G)
                    nc.gpsimd.tensor_mul(pprod3, f1t3, f2e3[:, :, col : col + W])
                    pool_prods.append(pprod)

                k = 0  # produced-shift index
                psumt = None

                def do_reduce(rhs_tile, rhs_off):
                    nonlocal k, psumt
                    j = k % PSH
                    if j == 0:
                        psumt = psumpool.tile([128, PSH * W], f32, tag="ps")
                    for g in range(G):
                        nc.tensor.matmul(
                            psumt[32 * g : 32 * (g + 1), j * W : (j + 1) * W],
                            wt[:],
                            rhs_tile[:, rhs_off + g * W : rhs_off + (g + 1) * W],
                            start=True,
                            stop=True,
                            tile_position=(0, 32 * g),
                        )
                    if j == PSH - 1 or k == TOP_CH - 1:
                        # PSUM f32 -> f16 staging (cast in the ACTIVATE).
                        nc.scalar.copy(
                            ob[:, (k - j) * W : (k + 1) * W],
                            psumt[:, 0 : (j + 1) * W],
                        )
                    k += 1

                for src_name, col0, m in CHUNKS:
                    src_t = f2o if src_name == "o" else f2e
                    # in1: [p, shift(stride 2), g(stride 544), w(stride 1)]
                    src4 = _ap(
                        src_t[:],
                        col0,
                        [[G * F2W, 128], [2, m], [F2W, G], [1, W]],
                    )
                    prod = prodpool.tile([128, 7 * G * W], f16, tag="prod")
                    prod4 = _ap(
                        prod[:], 0, [[7 * G * W, 128], [G * W, m], [W, G], [1, W]]
                    )
                    f1b = (
                        f1t[:]
                        .rearrange("p (s g w) -> p s g w", s=1, g=G)
                        .broadcast_to([128, m, G, W])
                    )
                    nc.vector.tensor_mul(prod4, f1b, src4)
                    for kl in range(m):
                        do_reduce(prod, kl * G * W)
                for pprod in pool_prods:
                    do_reduce(pprod, 0)

                # De-interleave staging (evens then odds) back to channel
                # order with two strided stores, one per HWDGE ring.
                dst_even = _ap(
                    out.ap(), ib * 128 * OBW, [[OBW, 128], [2 * W, 14], [1, W]]
                )
                nc.scalar.dma_start(dst_even, ob[:, 0 : 14 * W])
                dst_odd = _ap(
                    out.ap(), ib * 128 * OBW + W, [[OBW, 128], [2 * W, 13], [1, W]]
                )
                nc.sync.dma_start(dst_odd, ob[:, 14 * W : OBW])
    nc.compile()
    return nc


def _prep_inputs(l16: np.ndarray, r16: np.ndarray):
    """Build per-core block images. l16/r16: [C, H, W] f16 for one core."""
    # [16, 256, 512] -> [g4, c4, b8, h32, w512] -> (b, c, h, g, w)
    l5 = l16.reshape(G, G, NBLK, HB, W).transpose(2, 1, 3, 0, 4)
    l_blk = np.ascontiguousarray(l5).reshape(NBLK, 128, G * W)
    r5 = r16.reshape(G, G, NBLK, HB, W).transpose(2, 1, 3, 0, 4)  # b c h g w
    r_e = np.zeros((NBLK, G, HB, G, F2W), np.float16)
    r_e[:, :, :, :, PADL : PADL + W] = r5
    r_o = np.zeros((NBLK, G, HB, G, F2W), np.float16)
    r_o[:, :, :, :, PADL - 1 : PADL - 1 + W] = r5
    return (
        l_blk,
        r_e.reshape(NBLK, 128, G * F2W),
        r_o.reshape(NBLK, 128, G * F2W),
    )


def _unpack_output(o_blk: np.ndarray) -> np.ndarray:
    """[NBLK, 128, 27*512] f16 block layout -> [108, 256, 512] f32."""
    o5 = o_blk.reshape(NBLK, G, HB, TOP_CH, W).transpose(1, 3, 0, 2, 4)
    return np.ascontiguousarray(o5, dtype=np.float32).reshape(OUT_CH, H, W)


def kernel(l_in: np.ndarray, r_in: np.ndarray) -> np.ndarray:
    assert l_in.shape == (N, C, H, W) and r_in.shape == (N, C, H, W)
    l16 = np.asarray(l_in, dtype=np.float16)
    r16 = np.asarray(r_in, dtype=np.float16)

    if "nc" not in _CACHED:
        _CACHED["nc"] = _build_program()
    nc = _CACHED["nc"]

    w_np = _reduction_weights()
    in_maps = []
    for i in range(N):
        l_blk, re_, ro_ = _prep_inputs(l16[i], r16[i])
        in_maps.append({"l_blk": l_blk, "r_e": re_, "r_o": ro_, "w_red": w_np})

    trace = bool(int(os.environ.get("CORR_KERNEL_TRACE", "0")))
    kwargs = {}
    tdir = os.environ.get("CORR_KERNEL_TRACE_DIR")
    if trace and tdir:
        os.makedirs(tdir, exist_ok=True)
        kwargs["tmpdir"] = tdir
    res = bass_utils.run_bass_kernel_spmd(
        nc, in_maps, core_ids=list(range(N)), trace=trace, **kwargs
    )
    _CACHED["last_result"] = res
    return np.stack([_unpack_output(res.results[i]["out"]) for i in range(N)], axis=0)


# revision 11
# speedup vs baseline: 1.5334x; 1.5334x over previous
"""Corr1d-x-group cost-volume kernel for Trainium2 (8 NeuronCores, SPMD).

Data-parallel over batch N=8: core i processes batch i.

Per core (inputs [16, 256, 512], output [108, 256, 512]):
  out[g*27+ch, h, w] = 0.25 * sum_c f1[g*4+c, h, w] * f2[g*4+c, h, w+ch-23]
with zero padding outside w in [0, 512).

v3 design (bottleneck: the shift-product stream on VectorE, capped at DVE
2x_1P mode ~= (58 + FD/2)/0.96 ns per op):
  - Host pre-casts inputs to f16 and PRE-ARRANGES them into per-block SBUF
    images (zero pads + dual parity copies of f2 baked in), so every load
    is a contiguous DMA. Output stored f16; host un-permutes and upcasts.
  - SBUF layout per 32-row h-block: partitions = (c4, h32) = 128.
  - DVE work fused into 4 chunk ops per block: each computes 6-7 same-parity
    shifts x 4 groups in ONE tensor_tensor (overlapping strided view of the
    padded f2 tile; f1 broadcast along the shift dim with stride 0).
    Dual parity copies keep every row start 4B-aligned for 2x_1P mode.
  - Channel reduction via TensorE: 4 col-tiled matmuls per shift with a
    constant [128, 32] block-diagonal 0.25 weight -> fp32 PSUM; 4 shifts
    share one 4-bank PSUM tile.
  - ScalarE evacuates PSUM -> f16 SBUF staging (cast in ACTIVATE). Staging
    is in shift-production order (evens then odds); two strided stores per
    block de-interleave channels back to DRAM order.
"""

import os
import numpy as np

import concourse.bass as bass
import concourse.bacc as bacc
import concourse.mybir as mybir
import concourse.tile as tile
from concourse import bass_utils

N, C, H, W = 8, 16, 256, 512
G = 4
TOP_CH = 27
OUT_CH = G * TOP_CH  # 108
HB = 32  # h rows per block; 4 channels * 32 rows = 128 partitions
NBLK = H // HB
PADL = 24  # f2 even tile: column = w + PADL within each 544-col group chunk
F2W = PADL + W + 8  # 544 columns per group chunk
PSH = 4  # shifts per PSUM tile (4 banks); last tile holds 3

# Shift production order: even channels (from the odd-parity tile, aligned
# starts 0,2,..,26) then odd channels (from the even-parity tile, starts
# 2,4,..,26). Produced index k -> channel: k<14 -> 2k; k>=14 -> 2(k-14)+1.
# DVE chunks: (source, start_col, n_shifts); source 'o' = f2o, 'e' = f2e.
# NOTE: offloading shifts to GpSimd was tried and is a large regression --
# GpSimd shares the second DVE SBUF port, so Pool tensor ops and 2-port
# DVE tensor_tensor serialize against each other (measured ~1.8x TT dur).
CHUNKS = [("o", 0, 7), ("o", 14, 7), ("e", 2, 7), ("e", 16, 6)]

_CACHED = {}


def _reduction_weights() -> np.ndarray:
    # lhsT [K=(c, h32)=128, M=h32=32]: sums the 4 channels of a group and
    # applies the 1/sumelems scale.
    w = np.zeros((128, 32), np.float16)
    for c in range(G):
        for hh in range(HB):
            w[c * HB + hh, hh] = 0.25
    return w


def _ap(base, offset_elems, dims):
    """Raw AP on base's tensor: dims = [[stride, count], ...] in elements."""
    return bass.AP(tensor=base.tensor, offset=base.offset + offset_elems, ap=dims)


def _build_program() -> bass.Bass:
    # Bacc (not raw Bass): its compile() splits multi-sem sync waits, which
    # TRN2 hardware limits to one per instruction.
    nc = bacc.Bacc(
        "TRN2",
        target_bir_lowering=False,
        debug=False,
        enable_asserts=False,
        num_devices=N,
    )
    f16 = mybir.dt.float16
    f32 = mybir.dt.float32

    l_blk = nc.dram_tensor("l_blk", [NBLK, 128, G * W], f16, kind="ExternalInput")
    r_e = nc.dram_tensor("r_e", [NBLK, 128, G * F2W], f16, kind="ExternalInput")
    r_o = nc.dram_tensor("r_o", [NBLK, 128, G * F2W], f16, kind="ExternalInput")
    w_red = nc.dram_tensor("w_red", [128, 32], f16, kind="ExternalInput")
    # out[ib][g*32+h][ch*512+w] = out[g*27+ch, ib*32+h, w] (channel order).
    out = nc.dram_tensor("out", [NBLK, 128, TOP_CH * W], f16, kind="ExternalOutput")
    OBW = TOP_CH * W  # 13824 staging columns per partition

    with tile.TileContext(nc) as tc:
        with (
            tc.tile_pool(name="wpool", bufs=1) as wpool,
            tc.tile_pool(name="inpool", bufs=2) as inpool,
            tc.tile_pool(name="prodpool", bufs=3) as prodpool,
            tc.tile_pool(name="obpool", bufs=2) as obpool,
            tc.tile_pool(name="psumpool", bufs=2, space="PSUM") as psumpool,
        ):
            wt = wpool.tile([128, 32], f16)
            nc.sync.dma_start(wt[:], w_red[:])

            for ib in range(NBLK):
                # f2e (only needed by the 3rd chunk onward) rides the idle
                # Activation HWDGE ring; the SP ring serves f1 + f2o so the
                # first chunk starts as early as possible.
                f1t = inpool.tile([128, G * W], f16, tag="f1t")
                nc.sync.dma_start(f1t[:], l_blk.ap()[ib : ib + 1])
                f2o = inpool.tile([128, G * F2W], f16, tag="f2o")
                nc.sync.dma_start(f2o[:], r_o.ap()[ib : ib + 1])
                f2e = inpool.tile([128, G * F2W], f16, tag="f2e")
                nc.scalar.dma_start(f2e[:], r_e.ap()[ib : ib + 1])

                ob = obpool.tile([128, OBW], f16, tag="ob")

                k = 0  # produced-shift index
                psumt = None

                def do_reduce(rhs_tile, rhs_off):
                    nonlocal k, psumt
                    j = k % PSH
                    if j == 0:
                        psumt = psumpool.tile([128, PSH * W], f32, tag="ps")
                    for g in range(G):
                        nc.tensor.matmul(
                            psumt[32 * g : 32 * (g + 1), j * W : (j + 1) * W],
                            wt[:],
                            rhs_tile[:, rhs_off + g * W : rhs_off + (g + 1) * W],
                            start=True,
                            stop=True,
                            tile_position=(0, 32 * g),
                        )
                    if j == PSH - 1 or k == TOP_CH - 1:
                        # PSUM f32 -> f16 staging (cast in the ACTIVATE).
                        nc.scalar.copy(
                            ob[:, (k - j) * W : (k + 1) * W],
                            psumt[:, 0 : (j + 1) * W],
                        )
                    k += 1

                for src_name, col0, m in CHUNKS:
                    src_t = f2o if src_name == "o" else f2e
                    # in1: [p, shift(stride 2), g(stride 544), w(stride 1)]
                    src4 = _ap(
                        src_t[:],
                        col0,
                        [[G * F2W, 128], [2, m], [F2W, G], [1, W]],
                    )
                    prod = prodpool.tile([128, 7 * G * W], f16, tag="prod")
                    prod4 = _ap(
                        prod[:], 0, [[7 * G * W, 128], [G * W, m], [W, G], [1, W]]
                    )
                    f1b = (
                        f1t[:]
                        .rearrange("p (s g w) -> p s g w", s=1, g=G)
                        .broadcast_to([128, m, G, W])
                    )
                    nc.vector.tensor_mul(prod4, f1b, src4)
                    for kl in range(m):
                        do_reduce(prod, kl * G * W)

                # De-interleave staging (evens then odds) back to channel
                # order with two strided stores, one per HWDGE ring.
                dst_even = _ap(
                    out.ap(), ib * 128 * OBW, [[OBW, 128], [2 * W, 14], [1, W]]
                )
                nc.scalar.dma_start(dst_even, ob[:, 0 : 14 * W])
                dst_odd = _ap(
                    out.ap(), ib * 128 * OBW + W, [[OBW, 128], [2 * W, 13], [1, W]]
                )
                nc.sync.dma_start(dst_odd, ob[:, 14 * W : OBW])
    nc.compile()
    return nc


def _prep_inputs(l16: np.ndarray, r16: np.ndarray):
    """Build per-core block images. l16/r16: [C, H, W] f16 for one core."""
    # [16, 256, 512] -> [g4, c4, b8, h32, w512] -> (b, c, h, g, w)
    l5 = l16.reshape(G, G, NBLK, HB, W).transpose(2, 1, 3, 0, 4)
    l_blk = np.ascontiguousarray(l5).reshape(NBLK, 128, G * W)
    r5 = r16.reshape(G, G, NBLK, HB, W).transpose(2, 1, 3, 0, 4)  # b c h g w
    r_e = np.zeros((NBLK, G, HB, G, F2W), np.float16)
    r_e[:, :, :, :, PADL : PADL + W] = r5
    r_o = np.zeros((NBLK, G, HB, G, F2W), np.float16)
    r_o[:, :, :, :, PADL - 1 : PADL - 1 + W] = r5
    return (
        l_blk,
        r_e.reshape(NBLK, 128, G * F2W),
        r_o.reshape(NBLK, 128, G * F2W),
    )


def _unpack_output(o_blk: np.ndarray) -> np.ndarray:
    """[NBLK, 128, 27*512] f16 block layout -> [108, 256, 512] f32."""
    o5 = o_blk.reshape(NBLK, G, HB, TOP_CH, W).transpose(1, 3, 0, 2, 4)
    return np.ascontiguousarray(o5, dtype=np.float32).reshape(OUT_CH, H, W)


def kernel(l_in: np.ndarray, r_in: np.ndarray) -> np.ndarray:
    assert l_in.shape == (N, C, H, W) and r_in.shape == (N, C, H, W)
    l16 = np.asarray(l_in, dtype=np.float16)
    r16 = np.asarray(r_in, dtype=np.float16)

    if "nc" not in _CACHED:
        _CACHED["nc"] = _build_program()
    nc = _CACHED["nc"]

    w_np = _reduction_weights()
    in_maps = []
    for i in range(N):
        l_blk, re_, ro_ = _prep_inputs(l16[i], r16[i])
        in_maps.append({"l_blk": l_blk, "r_e": re_, "r_o": ro_, "w_red": w_np})

    trace = bool(int(os.environ.get("CORR_KERNEL_TRACE", "0")))
    kwargs = {}
    tdir = os.environ.get("CORR_KERNEL_TRACE_DIR")
    if trace and tdir:
        os.makedirs(tdir, exist_ok=True)
        kwargs["tmpdir"] = tdir
    res = bass_utils.run_bass_kernel_spmd(
        nc, in_maps, core_ids=list(range(N)), trace=trace, **kwargs
    )
    _CACHED["last_result"] = res
    return np.stack([_unpack_output(res.results[i]["out"]) for i in range(N)], axis=0)


# revision 15
# speedup vs baseline: 1.5477x; 1.0093x over previous
"""Corr1d-x-group cost-volume kernel for Trainium2 (8 NeuronCores, SPMD).

Data-parallel over batch N=8: core i processes batch i.

Per core (inputs [16, 256, 512], output [108, 256, 512]):
  out[g*27+ch, h, w] = 0.25 * sum_c f1[g*4+c, h, w] * f2[g*4+c, h, w+ch-23]
with zero padding outside w in [0, 512).

v3 design (bottleneck: the shift-product stream on VectorE, capped at DVE
2x_1P mode ~= (58 + FD/2)/0.96 ns per op):
  - Host pre-casts inputs to f16 and PRE-ARRANGES them into per-block SBUF
    images (zero pads + dual parity copies of f2 baked in), so every load
    is a contiguous DMA. Output stored f16; host un-permutes and upcasts.
  - SBUF layout per 32-row h-block: partitions = (c4, h32) = 128.
  - DVE work fused into 4 chunk ops per block: each computes 6-7 same-parity
    shifts x 4 groups in ONE tensor_tensor (overlapping strided view of the
    padded f2 tile; f1 broadcast along the shift dim with stride 0).
    Dual parity copies keep every row start 4B-aligned for 2x_1P mode.
  - Channel reduction via TensorE: 4 col-tiled matmuls per shift with a
    constant [128, 32] block-diagonal 0.25 weight -> fp32 PSUM; 4 shifts
    share one 4-bank PSUM tile.
  - ScalarE evacuates PSUM -> f16 SBUF staging (cast in ACTIVATE). Staging
    is in shift-production order (evens then odds); two strided stores per
    block de-interleave channels back to DRAM order.
"""

import os
import numpy as np

import concourse.bass as bass
import concourse.bacc as bacc
import concourse.mybir as mybir
import concourse.tile as tile
from concourse import bass_utils

N, C, H, W = 8, 16, 256, 512
G = 4
TOP_CH = 27
OUT_CH = G * TOP_CH  # 108
HB = 32  # h rows per block; 4 channels * 32 rows = 128 partitions
NBLK = H // HB
PADL = 24  # f2 even tile: column = w + PADL within each 544-col group chunk
F2W = PADL + W + 8  # 544 columns per group chunk
PSH = 4  # shifts per PSUM tile (4 banks); last tile holds 3

# Shift production order: even channels (from the odd-parity tile, aligned
# starts 0,2,..,26) then odd channels (from the even-parity tile, starts
# 2,4,..,26). Produced index k -> channel: k<14 -> 2k; k>=14 -> 2(k-14)+1.
# DVE chunks: (source, start_col, n_shifts); source 'o' = f2o, 'e' = f2e.
# NOTE: offloading shifts to GpSimd was tried and is a large regression --
# GpSimd shares the second DVE SBUF port, so Pool tensor ops and 2-port
# DVE tensor_tensor serialize against each other (measured ~1.8x TT dur).
CHUNKS = [("o", 0, 7), ("o", 14, 7), ("e", 2, 7), ("e", 16, 6)]
# Last block: split the final chunk so the matmul/evac/store pipeline can
# drain during the remaining TTs, shortening the kernel tail.
CHUNKS_LAST = [("o", 0, 7), ("o", 14, 7), ("e", 2, 7), ("e", 16, 3), ("e", 22, 3)]

_CACHED = {}


def _reduction_weights() -> np.ndarray:
    # lhsT [K=(c, h32)=128, M=h32=32]: sums the 4 channels of a group and
    # applies the 1/sumelems scale.
    w = np.zeros((128, 32), np.float16)
    for c in range(G):
        for hh in range(HB):
            w[c * HB + hh, hh] = 0.25
    return w


def _ap(base, offset_elems, dims):
    """Raw AP on base's tensor: dims = [[stride, count], ...] in elements."""
    return bass.AP(tensor=base.tensor, offset=base.offset + offset_elems, ap=dims)


def _build_program() -> bass.Bass:
    # Bacc (not raw Bass): its compile() splits multi-sem sync waits, which
    # TRN2 hardware limits to one per instruction.
    nc = bacc.Bacc(
        "TRN2",
        target_bir_lowering=False,
        debug=False,
        enable_asserts=False,
        num_devices=N,
    )
    f16 = mybir.dt.float16
    f32 = mybir.dt.float32

    l_blk = nc.dram_tensor("l_blk", [NBLK, 128, G * W], f16, kind="ExternalInput")
    r_e = nc.dram_tensor("r_e", [NBLK, 128, G * F2W], f16, kind="ExternalInput")
    r_o = nc.dram_tensor("r_o", [NBLK, 128, G * F2W], f16, kind="ExternalInput")
    w_red = nc.dram_tensor("w_red", [128, 32], f16, kind="ExternalInput")
    # out[ib][g*32+h][ch*512+w] = out[g*27+ch, ib*32+h, w] (channel order).
    out = nc.dram_tensor("out", [NBLK, 128, TOP_CH * W], f16, kind="ExternalOutput")
    OBW = TOP_CH * W  # 13824 staging columns per partition

    with tile.TileContext(nc) as tc:
        with (
            tc.tile_pool(name="wpool", bufs=1) as wpool,
            tc.tile_pool(name="inpool", bufs=2) as inpool,
            tc.tile_pool(name="prodpool", bufs=3) as prodpool,
            tc.tile_pool(name="obpool", bufs=2) as obpool,
            tc.tile_pool(name="psumpool", bufs=2, space="PSUM") as psumpool,
        ):
            wt = wpool.tile([128, 32], f16)
            nc.sync.dma_start(wt[:], w_red[:])

            for ib in range(NBLK):
                # f2e (only needed by the 3rd chunk onward) rides the idle
                # Activation HWDGE ring; the SP ring serves f1 + f2o so the
                # first chunk starts as early as possible.
                f1t = inpool.tile([128, G * W], f16, tag="f1t")
                nc.sync.dma_start(f1t[:], l_blk.ap()[ib : ib + 1])
                f2o = inpool.tile([128, G * F2W], f16, tag="f2o")
                nc.sync.dma_start(f2o[:], r_o.ap()[ib : ib + 1])
                f2e = inpool.tile([128, G * F2W], f16, tag="f2e")
                nc.scalar.dma_start(f2e[:], r_e.ap()[ib : ib + 1])

                ob = obpool.tile([128, OBW], f16, tag="ob")

                k = 0  # produced-shift index
                psumt = None

                def do_reduce(rhs_tile, rhs_off):
                    nonlocal k, psumt
                    j = k % PSH
                    if j == 0:
                        psumt = psumpool.tile([128, PSH * W], f32, tag="ps")
                    for g in range(G):
                        nc.tensor.matmul(
                            psumt[32 * g : 32 * (g + 1), j * W : (j + 1) * W],
                            wt[:],
                            rhs_tile[:, rhs_off + g * W : rhs_off + (g + 1) * W],
                            start=True,
                            stop=True,
                            tile_position=(0, 32 * g),
                        )
                    if j == PSH - 1 or k == TOP_CH - 1:
                        # PSUM f32 -> f16 staging (cast in the ACTIVATE).
                        nc.scalar.copy(
                            ob[:, (k - j) * W : (k + 1) * W],
                            psumt[:, 0 : (j + 1) * W],
                        )
                        if k == 15:
                            # Even-channel half (produced k 0..13) is fully
                            # staged; store it early off the tail.
                            dst_even = _ap(
                                out.ap(),
                                ib * 128 * OBW,
                                [[OBW, 128], [2 * W, 14], [1, W]],
                            )
                            nc.scalar.dma_start(dst_even, ob[:, 0 : 14 * W])
                    k += 1

                blk_chunks = CHUNKS if ib < NBLK - 1 else CHUNKS_LAST
                for src_name, col0, m in blk_chunks:
                    src_t = f2o if src_name == "o" else f2e
                    # in1: [p, shift(stride 2), g(stride 544), w(stride 1)]
                    src4 = _ap(
                        src_t[:],
                        col0,
                        [[G * F2W, 128], [2, m], [F2W, G], [1, W]],
                    )
                    prod = prodpool.tile([128, 7 * G * W], f16, tag="prod")
                    prod4 = _ap(
                        prod[:], 0, [[7 * G * W, 128], [G * W, m], [W, G], [1, W]]
                    )
                    f1b = (
                        f1t[:]
                        .rearrange("p (s g w) -> p s g w", s=1, g=G)
                        .broadcast_to([128, m, G, W])
                    )
                    nc.vector.tensor_mul(prod4, f1b, src4)
                    for kl in range(m):
                        do_reduce(prod, kl * G * W)

                # Odd-channel half (even half was stored early, above).
                dst_odd = _ap(
                    out.ap(), ib * 128 * OBW + W, [[OBW, 128], [2 * W, 13], [1, W]]
                )
                nc.sync.dma_start(dst_odd, ob[:, 14 * W : OBW])
    nc.compile()
    return nc


def _prep_inputs(l16: np.ndarray, r16: np.ndarray):
    """Build per-core block images. l16/r16: [C, H, W] f16 for one core."""
    # [16, 256, 512] -> [g4, c4, b8, h32, w512] -> (b, c, h, g, w)
    l5 = l16.reshape(G, G, NBLK, HB, W).transpose(2, 1, 3, 0, 4)
    l_blk = np.ascontiguousarray(l5).reshape(NBLK, 128, G * W)
    r5 = r16.reshape(G, G, NBLK, HB, W).transpose(2, 1, 3, 0, 4)  # b c h g w
    r_e = np.zeros((NBLK, G, HB, G, F2W), np.float16)
    r_e[:, :, :, :, PADL : PADL + W] = r5
    r_o = np.zeros((NBLK, G, HB, G, F2W), np.float16)
    r_o[:, :, :, :, PADL - 1 : PADL - 1 + W] = r5
    return (
        l_blk,
        r_e.reshape(NBLK, 128, G * F2W),
        r_o.reshape(NBLK, 128, G * F2W),
    )


def _unpack_output(o_blk: np.ndarray) -> np.ndarray:
    """[NBLK, 128, 27*512] f16 block layout -> [108, 256, 512] f32."""
    o5 = o_blk.reshape(NBLK, G, HB, TOP_CH, W).transpose(1, 3, 0, 2, 4)
    return np.ascontiguousarray(o5, dtype=np.float32).reshape(OUT_CH, H, W)


def kernel(l_in: np.ndarray, r_in: np.ndarray) -> np.ndarray:
    assert l_in.shape == (N, C, H, W) and r_in.shape == (N, C, H, W)
    l16 = np.asarray(l_in, dtype=np.float16)
    r16 = np.asarray(r_in, dtype=np.float16)

    if "nc" not in _CACHED:
        _CACHED["nc"] = _build_program()
    nc = _CACHED["nc"]

    w_np = _reduction_weights()
    in_maps = []
    for i in range(N):
        l_blk, re_, ro_ = _prep_inputs(l16[i], r16[i])
        in_maps.append({"l_blk": l_blk, "r_e": re_, "r_o": ro_, "w_red": w_np})

    trace = bool(int(os.environ.get("CORR_KERNEL_TRACE", "0")))
    kwargs = {}
    tdir = os.environ.get("CORR_KERNEL_TRACE_DIR")
    if trace and tdir:
        os.makedirs(tdir, exist_ok=True)
        kwargs["tmpdir"] = tdir
    res = bass_utils.run_bass_kernel_spmd(
        nc, in_maps, core_ids=list(range(N)), trace=trace, **kwargs
    )
    _CACHED["last_result"] = res
    return np.stack([_unpack_output(res.results[i]["out"]) for i in range(N)], axis=0)


# revision 16
# speedup vs baseline: 1.5708x; 1.0149x over previous
"""Corr1d-x-group cost-volume kernel for Trainium2 (8 NeuronCores, SPMD).

Data-parallel over batch N=8: core i processes batch i.

Per core (inputs [16, 256, 512], output [108, 256, 512]):
  out[g*27+ch, h, w] = 0.25 * sum_c f1[g*4+c, h, w] * f2[g*4+c, h, w+ch-23]
with zero padding outside w in [0, 512).

v3 design (bottleneck: the shift-product stream on VectorE, capped at DVE
2x_1P mode ~= (58 + FD/2)/0.96 ns per op):
  - Host pre-casts inputs to f16 and PRE-ARRANGES them into per-block SBUF
    images (zero pads + dual parity copies of f2 baked in), so every load
    is a contiguous DMA. Output stored f16; host un-permutes and upcasts.
  - SBUF layout per 32-row h-block: partitions = (c4, h32) = 128.
  - DVE work fused into 4 chunk ops per block: each computes 6-7 same-parity
    shifts x 4 groups in ONE tensor_tensor (overlapping strided view of the
    padded f2 tile; f1 broadcast along the shift dim with stride 0).
    Dual parity copies keep every row start 4B-aligned for 2x_1P mode.
  - Channel reduction via TensorE: 4 col-tiled matmuls per shift with a
    constant [128, 32] block-diagonal 0.25 weight -> fp32 PSUM; 4 shifts
    share one 4-bank PSUM tile.
  - ScalarE evacuates PSUM -> f16 SBUF staging (cast in ACTIVATE). Staging
    is in shift-production order (evens then odds); two strided stores per
    block de-interleave channels back to DRAM order.
"""

import os
import numpy as np

import concourse.bass as bass
import concourse.bacc as bacc
import concourse.mybir as mybir
import concourse.tile as tile
from concourse import bass_utils

N, C, H, W = 8, 16, 256, 512
G = 4
TOP_CH = 27
OUT_CH = G * TOP_CH  # 108
HB = 32  # h rows per block; 4 channels * 32 rows = 128 partitions
NBLK = H // HB
PADL = 24  # f2 even tile: column = w + PADL within each 544-col group chunk
F2W = PADL + W + 8  # 544 columns per group chunk
PSH = 4  # shifts per PSUM tile (4 banks); last tile holds 3

# Shift production order: even channels (from the odd-parity tile, aligned
# starts 0,2,..,26) then odd channels (from the even-parity tile, starts
# 2,4,..,26). Produced index k -> channel: k<14 -> 2k; k>=14 -> 2(k-14)+1.
# DVE chunks: (source, start_col, n_shifts); source 'o' = f2o, 'e' = f2e.
# NOTE: offloading shifts to GpSimd was tried and is a large regression --
# GpSimd shares the second DVE SBUF port, so Pool tensor ops and 2-port
# DVE tensor_tensor serialize against each other (measured ~1.8x TT dur).
CHUNKS = [("o", 0, 7), ("o", 14, 7), ("e", 2, 7), ("e", 16, 6)]
# Last block: split the final chunk so the matmul/evac/store pipeline can
# drain during the remaining TTs, shortening the kernel tail.
CHUNKS_LAST = [("o", 0, 7), ("o", 14, 7), ("e", 2, 7), ("e", 16, 3), ("e", 22, 3)]

_CACHED = {}


def _reduction_weights() -> np.ndarray:
    # lhsT [K=(c, h32)=128, M=h32=32]: sums the 4 channels of a group and
    # applies the 1/sumelems scale.
    w = np.zeros((128, 32), np.float16)
    for c in range(G):
        for hh in range(HB):
            w[c * HB + hh, hh] = 0.25
    return w


def _ap(base, offset_elems, dims):
    """Raw AP on base's tensor: dims = [[stride, count], ...] in elements."""
    return bass.AP(tensor=base.tensor, offset=base.offset + offset_elems, ap=dims)


def _build_program() -> bass.Bass:
    # Bacc (not raw Bass): its compile() splits multi-sem sync waits, which
    # TRN2 hardware limits to one per instruction.
    nc = bacc.Bacc(
        "TRN2",
        target_bir_lowering=False,
        debug=False,
        enable_asserts=False,
        num_devices=N,
    )
    f16 = mybir.dt.float16
    f32 = mybir.dt.float32

    l_blk = nc.dram_tensor("l_blk", [NBLK, 128, G * W], f16, kind="ExternalInput")
    r_e = nc.dram_tensor("r_e", [NBLK, 128, G * F2W], f16, kind="ExternalInput")
    r_o = nc.dram_tensor("r_o", [NBLK, 128, G * F2W], f16, kind="ExternalInput")
    w_red = nc.dram_tensor("w_red", [128, 32], f16, kind="ExternalInput")
    # out[ib][g*32+h][ch*512+w] = out[g*27+ch, ib*32+h, w] (channel order).
    out = nc.dram_tensor("out", [NBLK, 128, TOP_CH * W], f16, kind="ExternalOutput")
    OBW = TOP_CH * W  # 13824 staging columns per partition

    with tile.TileContext(nc) as tc:
        with (
            tc.tile_pool(name="wpool", bufs=1) as wpool,
            tc.tile_pool(name="inpool", bufs=2) as inpool,
            tc.tile_pool(name="prodpool", bufs=3) as prodpool,
            tc.tile_pool(name="obpool", bufs=2) as obpool,
            tc.tile_pool(name="psumpool", bufs=2, space="PSUM") as psumpool,
        ):
            wt = wpool.tile([128, 32], f16)

            for ib in range(NBLK):
                # The first chunk needs f1 + f2o: run them on DIFFERENT HWDGE
                # rings (SP / Activation) so they transfer concurrently; f2e
                # (only needed by the 3rd chunk) follows f1 on the SP ring.
                f1t = inpool.tile([128, G * W], f16, tag="f1t")
                nc.sync.dma_start(f1t[:], l_blk.ap()[ib : ib + 1])
                f2o = inpool.tile([128, G * F2W], f16, tag="f2o")
                nc.scalar.dma_start(f2o[:], r_o.ap()[ib : ib + 1])
                f2e = inpool.tile([128, G * F2W], f16, tag="f2e")
                nc.sync.dma_start(f2e[:], r_e.ap()[ib : ib + 1])
                if ib == 0:
                    # Weights are first needed by the first matmul (~17us in);
                    # keep them off the critical first-chunk load path.
                    nc.sync.dma_start(wt[:], w_red[:])

                ob = obpool.tile([128, OBW], f16, tag="ob")

                k = 0  # produced-shift index
                psumt = None

                def do_reduce(rhs_tile, rhs_off):
                    nonlocal k, psumt
                    j = k % PSH
                    if j == 0:
                        psumt = psumpool.tile([128, PSH * W], f32, tag="ps")
                    for g in range(G):
                        nc.tensor.matmul(
                            psumt[32 * g : 32 * (g + 1), j * W : (j + 1) * W],
                            wt[:],
                            rhs_tile[:, rhs_off + g * W : rhs_off + (g + 1) * W],
                            start=True,
                            stop=True,
                            tile_position=(0, 32 * g),
                        )
                    if j == PSH - 1 or k == TOP_CH - 1:
                        # PSUM f32 -> f16 staging (cast in the ACTIVATE).
                        nc.scalar.copy(
                            ob[:, (k - j) * W : (k + 1) * W],
                            psumt[:, 0 : (j + 1) * W],
                        )
                        if k == 15:
                            # Even-channel half (produced k 0..13) is fully
                            # staged; store it early off the tail.
                            dst_even = _ap(
                                out.ap(),
                                ib * 128 * OBW,
                                [[OBW, 128], [2 * W, 14], [1, W]],
                            )
                            nc.scalar.dma_start(dst_even, ob[:, 0 : 14 * W])
                    k += 1

                blk_chunks = CHUNKS if ib < NBLK - 1 else CHUNKS_LAST
                for src_name, col0, m in blk_chunks:
                    src_t = f2o if src_name == "o" else f2e
                    # in1: [p, shift(stride 2), g(stride 544), w(stride 1)]
                    src4 = _ap(
                        src_t[:],
                        col0,
                        [[G * F2W, 128], [2, m], [F2W, G], [1, W]],
                    )
                    prod = prodpool.tile([128, 7 * G * W], f16, tag="prod")
                    prod4 = _ap(
                        prod[:], 0, [[7 * G * W, 128], [G * W, m], [W, G], [1, W]]
                    )
                    f1b = (
                        f1t[:]
                        .rearrange("p (s g w) -> p s g w", s=1, g=G)
                        .broadcast_to([128, m, G, W])
                    )
                    nc.vector.tensor_mul(prod4, f1b, src4)
                    for kl in range(m):
                        do_reduce(prod, kl * G * W)

                # Odd-channel half (even half was stored early, above).
                dst_odd = _ap(
                    out.ap(), ib * 128 * OBW + W, [[OBW, 128], [2 * W, 13], [1, W]]
                )
                nc.sync.dma_start(dst_odd, ob[:, 14 * W : OBW])
    nc.compile()
    return nc


def _prep_inputs(l16: np.ndarray, r16: np.ndarray):
    """Build per-core block images. l16/r16: [C, H, W] f16 for one core."""
    # [16, 256, 512] -> [g4, c4, b8, h32, w512] -> (b, c, h, g, w)
    l5 = l16.reshape(G, G, NBLK, HB, W).transpose(2, 1, 3, 0, 4)
    l_blk = np.ascontiguousarray(l5).reshape(NBLK, 128, G * W)
    r5 = r16.reshape(G, G, NBLK, HB, W).transpose(2, 1, 3, 0, 4)  # b c h g w
    r_e = np.zeros((NBLK, G, HB, G, F2W), np.float16)
    r_e[:, :, :, :, PADL : PADL + W] = r5
    r_o = np.zeros((NBLK, G, HB, G, F2W), np.float16)
    r_o[:, :, :, :, PADL - 1 : PADL - 1 + W] = r5
    return (
        l_blk,
        r_e.reshape(NBLK, 128, G * F2W),
        r_o.reshape(NBLK, 128, G * F2W),
    )


def _unpack_output(o_blk: np.ndarray) -> np.ndarray:
    """[NBLK, 128, 27*512] f16 block layout -> [108, 256, 512] f32."""
    o5 = o_blk.reshape(NBLK, G, HB, TOP_CH, W).transpose(1, 3, 0, 2, 4)
    return np.ascontiguousarray(o5, dtype=np.float32).reshape(OUT_CH, H, W)


def kernel(l_in: np.ndarray, r_in: np.ndarray) -> np.ndarray:
    assert l_in.shape == (N, C, H, W) and r_in.shape == (N, C, H, W)
    l16 = np.asarray(l_in, dtype=np.float16)
    r16 = np.asarray(r_in, dtype=np.float16)

    if "nc" not in _CACHED:
        _CACHED["nc"] = _build_program()
    nc = _CACHED["nc"]

    w_np = _reduction_weights()
    in_maps = []
    for i in range(N):
        l_blk, re_, ro_ = _prep_inputs(l16[i], r16[i])
        in_maps.append({"l_blk": l_blk, "r_e": re_, "r_o": ro_, "w_red": w_np})

    trace = bool(int(os.environ.get("CORR_KERNEL_TRACE", "0")))
    kwargs = {}
    tdir = os.environ.get("CORR_KERNEL_TRACE_DIR")
    if trace and tdir:
        os.makedirs(tdir, exist_ok=True)
        kwargs["tmpdir"] = tdir
    res = bass_utils.run_bass_kernel_spmd(
        nc, in_maps, core_ids=list(range(N)), trace=trace, **kwargs
    )
    _CACHED["last_result"] = res
    return np.stack([_unpack_output(res.results[i]["out"]) for i in range(N)], axis=0)
